# revision 15
# baseline (speedup 1.0000x reference)
"""Trainium2 Bass kernel for nn_DenseAttention_85074712199779.

reference computation (B=8, N=8192, D=512, H=8, DH=64):
    x   = hs * cos + rotate_half(hs) * sin          # RoPE
    q   = x @ W.T                                   # dense projection
    kv  = einsum('bnhd,bnhe->bhde', xh, xh)         # per-head K^T V
    out = einsum('bnhd,bhde->bnhe', qh, kv)         # per-head Q (K^T V)

Sharding: sequence dim N split across the 8 NeuronCores (1024 rows/core,
all batches).  kv needs a cross-core sum -> one small fp32 AllReduce
(1 MB) that overlaps the q-projection matmuls.  All matmuls run in bf16
(fp32 PSUM accumulation); inputs are pre-cast to bf16 on the host (also
halves HBM traffic).  Output is written bf16 and upcast on the host.

Device schedule per core (b = batch):
  - A1(b): DMA hs, RoPE on DVE/ACT (sign-folded sin table so rotate_half
    is a free-dim swap), stage2 kv = x_hp^T @ x_hp accumulated in PSUM
  - A2(b): PE transpose x -> xT, stage1 qT = WT.T @ xT -> resident bf16
    qT.  A2 for b0..b3 is interleaved (lag 2) to keep the PE dense;
    A2 for b4..b7 is deferred until after the AllReduce is issued so all
    kv partials finish as early as possible.
  - one AllReduce of kv, read back on the scalar DMA queue, cast on DVE
  - B(b): out[m,e] = qT_chunk.T @ blockdiag(kv head-pair), DMA out bf16.
"""

import sys

if "/opt/trn_rl_repo" not in sys.path:
    sys.path.insert(0, "/opt/trn_rl_repo")

import numpy as np
import ml_dtypes

import concourse.bass as bass
import concourse.mybir as mybir
import concourse.tile as tile
from concourse import bacc
from concourse.masks import make_identity
from concourse.tile_rust import add_dep_helper

B = 8          # batch
N = 8192       # sequence
D = 512        # hidden
NCORES = 8
R = N // NCORES          # rows per core (1024)
CH = R // 128            # 128-row chunks per batch per core (8)
MB_CH = 4                # chunks per m-block
NMB = CH // MB_CH        # m-blocks per batch (2)
LAG = 2                  # batches stage1 trails stage2 by (early batches)
NEARLY = 4               # how many batches get interleaved A2
GROUP = 4                # batches per kv AllReduce
BF16 = mybir.dt.bfloat16
F32 = mybir.dt.float32

_CACHE: dict = {}


def _build():
    nc = bacc.Bacc(trn_type="TRN2", num_devices=NCORES)

    hs_ext = nc.declare_dram_parameter("hs", [B, R, D], BF16, isOutput=False)
    cos_ext = nc.declare_dram_parameter("cosb", [R, D], BF16, isOutput=False)
    sin_ext = nc.declare_dram_parameter("sins", [R, D], BF16, isOutput=False)
    wt_ext = nc.declare_dram_parameter("wt", [D, D], BF16, isOutput=False)
    out_ext = nc.declare_dram_parameter("out", [B, R, D], BF16, isOutput=True)

    kv_part = nc.dram_tensor("kv_part", [B, 4, 2, 64, 64], F32)
    kv_red = nc.dram_tensor("kv_red", [B, 4, 2, 64, 64], F32, addr_space="Shared")

    rg = [list(range(NCORES))]

    with tile.TileContext(nc) as tc:
        with (
            tc.tile_pool(name="singles", bufs=1) as singles,
            tc.tile_pool(name="hs_p", bufs=2) as hs_p,
            tc.tile_pool(name="x_p", bufs=6) as x_p,
            tc.tile_pool(name="xt_p", bufs=2) as xt_p,
            tc.tile_pool(name="kvs_p", bufs=2) as kvs_p,
            tc.tile_pool(name="out_p", bufs=3) as out_p,
            tc.tile_pool(name="tp_ps", bufs=2, space="PSUM") as tp_ps,
            tc.tile_pool(name="q_ps", bufs=2, space="PSUM") as q_ps,
            tc.tile_pool(name="kv_ps", bufs=1, space="PSUM") as kv_ps,
            tc.tile_pool(name="o_ps", bufs=3, space="PSUM") as o_ps,
        ):
            import os as _os
            if _os.environ.get("K_WARM", "0") == "1":
                # communicator warm-up: absorbs the one-time cross-core barrier
                warm_in = nc.dram_tensor("warm_in", [1, 32], F32)
                warm_out = nc.dram_tensor("warm_out", [1, 32], F32,
                                          addr_space="Shared")
                warm_sb = singles.tile([1, 32], F32, name="warm_sb")
                warm_sb2 = singles.tile([1, 32], F32, name="warm_sb2")
                nc.vector.memset(warm_sb, 0.0)
                nc.sync.dma_start(out=warm_in[:, :], in_=warm_sb)
                wc = nc.gpsimd.collective_compute(
                    "AllReduce", mybir.AluOpType.add, replica_groups=rg,
                    ins=[warm_in[:, :]], outs=[warm_out[:, :]])
                wrb = nc.sync.dma_start(out=warm_sb2, in_=warm_out[:, :])
                add_dep_helper(wrb.ins, wc.ins, reason="warm readback")

            # ---- resident inputs; split across the two HWDGE queues so the
            # first batch's hs DMA isn't stuck behind all the constants ----
            cos_sb = singles.tile([128, CH, D], BF16, name="cos_sb")
            sin_sb = singles.tile([128, CH, D], BF16, name="sin_sb")
            cos_r = cos_ext.rearrange("(c p) d -> p c d", p=128)
            sin_r = sin_ext.rearrange("(c p) d -> p c d", p=128)
            # batch-0 first slice goes ahead of the bulk constants so RoPE
            # and the first kv matmuls start as early as possible
            hs0_t = hs_p.tile([128, MB_CH, D], BF16, name="hs_t")
            nc.sync.dma_start(
                out=hs0_t,
                in_=hs_ext[0].rearrange("(c p) d -> p c d", p=128)[:, 0:MB_CH, :])
            nc.sync.dma_start(out=cos_sb[:, 0:MB_CH, :], in_=cos_r[:, 0:MB_CH, :])
            nc.scalar.dma_start(out=sin_sb[:, 0:MB_CH, :], in_=sin_r[:, 0:MB_CH, :])
            nc.sync.dma_start(out=cos_sb[:, MB_CH:CH, :], in_=cos_r[:, MB_CH:CH, :])
            nc.scalar.dma_start(out=sin_sb[:, MB_CH:CH, :], in_=sin_r[:, MB_CH:CH, :])

            # WT strips (host-transposed): wt_sb[:, db, e] = W[e, db*128+p]
            wt_sb = singles.tile([128, 4, D], BF16, name="wt_sb")
            nc.scalar.dma_start(out=wt_sb,
                                in_=wt_ext.rearrange("(b p) e -> p b e", p=128))

            ident = singles.tile([128, 128], BF16, name="ident")
            make_identity(nc, ident)

            qT_sb = singles.tile([128, 4, B * R], BF16, name="qT_sb")
            kvblk = singles.tile([128, B, 4, 128], BF16, name="kvblk")
            nc.gpsimd.memset(kvblk, 0.0)

            x_tiles = {}
            kv_writers = []

            def emit_a1(b, hs_pre=None):
                """DMA + RoPE + stage2 kv accumulation for batch b."""
                x_t = x_p.tile([128, CH, D], BF16, name="x_t")
                x_tiles[b] = x_t
                kvp = kv_ps.tile([128, 4, 128], F32, name="kvp")
                hs_r = hs_ext[b].rearrange("(c p) d -> p c d", p=128)
                for mb in range(NMB):
                    cs = slice(mb * MB_CH, (mb + 1) * MB_CH)
                    if mb == 0 and hs_pre is not None:
                        hs_t = hs_pre
                    else:
                        hs_t = hs_p.tile([128, MB_CH, D], BF16, name="hs_t")
                        nc.sync.dma_start(out=hs_t, in_=hs_r[:, cs, :])
                    # RoPE: x = hs*cos + swap_half(hs)*sin_signed
                    nc.any.tensor_tensor(
                        x_t[:, cs, 0:256], hs_t[:, :, 256:512], sin_sb[:, cs, 0:256],
                        mybir.AluOpType.mult)
                    nc.any.tensor_tensor(
                        x_t[:, cs, 256:512], hs_t[:, :, 0:256], sin_sb[:, cs, 256:512],
                        mybir.AluOpType.mult)
                    nc.vector.tensor_tensor(hs_t, hs_t, cos_sb[:, cs, :],
                                            mybir.AluOpType.mult)
                    nc.vector.tensor_tensor(x_t[:, cs, :], x_t[:, cs, :], hs_t,
                                            mybir.AluOpType.add)
                    for cc in range(MB_CH):
                        c = mb * MB_CH + cc
                        for hp in range(4):
                            xs = x_t[:, c, hp * 128:(hp + 1) * 128]
                            nc.tensor.matmul(
                                kvp[:, hp, :], xs, xs,
                                start=(c == 0 and hp == 0),
                                stop=(c == CH - 1 and hp == 3))
                # evacuate kv partial (diagonal 64x64 blocks)
                kv_sb = kvs_p.tile([128, 4, 128], F32, name="kv_sb")
                nc.any.tensor_copy(out=kv_sb, in_=kvp)
                d0 = nc.sync.dma_start(
                    out=kv_part[b, :, 0].rearrange("h d e -> d h e"),
                    in_=kv_sb[0:64, :, 0:64])
                d1 = nc.sync.dma_start(
                    out=kv_part[b, :, 1].rearrange("h d e -> d h e"),
                    in_=kv_sb[64:128, :, 64:128])
                kv_writers.extend([d0, d1])

            def emit_a2(b):
                """PE transposes + stage1 qT for batch b."""
                x_t = x_tiles.pop(b)
                for mb in range(NMB):
                    xt_t = xt_p.tile([128, 4, MB_CH * 128], BF16, name="xt_t")
                    for cc in range(MB_CH):
                        c = mb * MB_CH + cc
                        tp = tp_ps.tile([128, 4, 128], BF16, name="tp")
                        for db in range(4):
                            nc.tensor.transpose(
                                tp[:, db, :], x_t[:, c, db * 128:(db + 1) * 128], ident)
                        nc.any.tensor_copy(out=xt_t[:, :, cc * 128:(cc + 1) * 128],
                                           in_=tp)
                    for eb in range(4):
                        qp = q_ps.tile([128, MB_CH * 128], F32, name="qp")
                        for db in range(4):
                            nc.tensor.matmul(
                                qp, wt_sb[:, db, eb * 128:(eb + 1) * 128],
                                xt_t[:, db, :],
                                start=(db == 0), stop=(db == 3))
                        nc.any.tensor_copy(
                            out=qT_sb[:, eb, b * R + mb * MB_CH * 128:
                                      b * R + (mb + 1) * MB_CH * 128],
                            in_=qp)

            colls = []

            def emit_allreduce(g0, g1):
                coll = nc.gpsimd.collective_compute(
                    "AllReduce", mybir.AluOpType.add, replica_groups=rg,
                    ins=[kv_part[g0:g1]], outs=[kv_red[g0:g1]])
                for w in kv_writers:
                    add_dep_helper(coll.ins, w.ins, reason="allreduce after kv dma")
                kv_writers.clear()
                colls.append((g0, g1, coll))

            def emit_readbacks():
                # SWDGE cast-DMAs (fp32 -> bf16) straight into the
                # block-diagonal tiles; emitted after both collectives so the
                # in-order gpsimd queue never delays an AllReduce trigger
                for g0, g1, coll in colls:
                    for bb in range(g0, g1):
                        r0 = nc.gpsimd.dma_start(
                            out=kvblk[0:64, bb, :, 0:64],
                            in_=kv_red[bb, :, 0].rearrange("h d e -> d h e"))
                        r1 = nc.gpsimd.dma_start(
                            out=kvblk[64:128, bb, :, 64:128],
                            in_=kv_red[bb, :, 1].rearrange("h d e -> d h e"))
                        add_dep_helper(r0.ins, coll.ins, reason="rb after allreduce")
                        add_dep_helper(r1.ins, coll.ins, reason="rb after allreduce")

            # ---------------- phase A ----------------
            for b in range(B):
                emit_a1(b, hs_pre=hs0_t if b == 0 else None)
                if LAG <= b < NEARLY + LAG:
                    emit_a2(b - LAG)
                if b == GROUP - 1:
                    emit_allreduce(0, GROUP)
                elif b == B - 1:
                    emit_allreduce(GROUP, B)
            emit_readbacks()

            # deferred stage1 for the late batches fills the AllReduce window,
            # interleaved with group-0 phase B so its matmuls + copies start
            # as soon as the first AllReduce lands instead of queueing behind
            # all the deferred stage1 work
            b_emitted = []

            def emit_b(b):
                b_emitted.append(b)
                out_r = out_ext[b].rearrange("(c p) d -> p c d", p=128)
                out_sb = out_p.tile([128, CH, D], BF16, name="out_sb")
                for c in range(CH):
                    op = o_ps.tile([128, D], F32, name="op")
                    for hp in range(4):
                        nc.tensor.matmul(
                            op[:, hp * 128:(hp + 1) * 128],
                            qT_sb[:, hp, b * R + c * 128:b * R + (c + 1) * 128],
                            kvblk[:, b, hp, :],
                            start=(hp == 0), stop=(hp == 3))
                    if c % 2 == 0:
                        nc.vector.tensor_copy(out=out_sb[:, c, :], in_=op)
                    else:
                        nc.scalar.copy(out=out_sb[:, c, :], in_=op)
                nc.sync.dma_start(out=out_r, in_=out_sb)

            emit_a2(NEARLY)
            emit_b(0)
            emit_a2(NEARLY + 1)
            emit_b(1)
            emit_a2(NEARLY + 2)
            emit_b(2)
            emit_a2(NEARLY + 3)
            emit_b(3)

            # ---------------- phase B (remaining batches) ----------------
            for b in range(4, B):
                out_r = out_ext[b].rearrange("(c p) d -> p c d", p=128)
                out_sb = out_p.tile([128, CH, D], BF16, name="out_sb")
                for c in range(CH):
                    op = o_ps.tile([128, D], F32, name="op")
                    for hp in range(4):
                        nc.tensor.matmul(
                            op[:, hp * 128:(hp + 1) * 128],
                            qT_sb[:, hp, b * R + c * 128:b * R + (c + 1) * 128],
                            kvblk[:, b, hp, :],
                            start=(hp == 0), stop=(hp == 3))
                    if c % 2 == 0:
                        nc.vector.tensor_copy(out=out_sb[:, c, :], in_=op)
                    else:
                        nc.scalar.copy(out=out_sb[:, c, :], in_=op)
                nc.sync.dma_start(out=out_r, in_=out_sb)

    nc.compile()
    return nc


def _prep_in_maps(hidden_states, W, cos, sin):
    bf16 = ml_dtypes.bfloat16
    hs = np.ascontiguousarray(hidden_states, dtype=np.float32)
    cos = np.asarray(cos, dtype=np.float32)
    sin = np.asarray(sin, dtype=np.float32)
    sin_signed = np.concatenate([-sin[:, : D // 2], sin[:, D // 2:]], axis=1)
    wt16 = np.ascontiguousarray(np.asarray(W, dtype=np.float32).T).astype(bf16)
    in_maps = []
    for c in range(NCORES):
        rows = slice(c * R, (c + 1) * R)
        in_maps.append({
            "hs": np.ascontiguousarray(hs[:, rows, :]).astype(bf16),
            "cosb": np.ascontiguousarray(cos[rows]).astype(bf16),
            "sins": np.ascontiguousarray(sin_signed[rows]).astype(bf16),
            "wt": wt16,
        })
    return in_maps


def _collect(results):
    out = np.empty((B, N, D), dtype=np.float32)
    for c in range(NCORES):
        out[:, c * R:(c + 1) * R, :] = results[c]["out"].astype(np.float32)
    return out


def kernel(hidden_states, W, cos, sin):
    from concourse.bass_utils import run_bass_kernel_spmd

    nc = _CACHE.get("nc")
    if nc is None:
        nc = _build()
        _CACHE["nc"] = nc

    in_maps = _prep_in_maps(hidden_states, W, cos, sin)
    res = run_bass_kernel_spmd(nc, in_maps, list(range(NCORES)))
    return _collect(res.results)


# revision 16
# speedup vs baseline: 1.0596x; 1.0596x over previous
"""Trainium2 Bass kernel for nn_DenseAttention_85074712199779.

reference computation (B=8, N=8192, D=512, H=8, DH=64):
    x   = hs * cos + rotate_half(hs) * sin          # RoPE
    q   = x @ W.T                                   # dense projection
    kv  = einsum('bnhd,bnhe->bhde', xh, xh)         # per-head K^T V
    out = einsum('bnhd,bhde->bnhe', qh, kv)         # per-head Q (K^T V)

Sharding: sequence dim N split across the 8 NeuronCores (1024 rows/core,
all batches).  kv needs a cross-core sum -> one small fp32 AllReduce
(1 MB) that overlaps the q-projection matmuls.  All matmuls run in bf16
(fp32 PSUM accumulation); inputs are pre-cast to bf16 on the host (also
halves HBM traffic).  Output is written bf16 and upcast on the host.

Device schedule per core (b = batch):
  - A1(b): DMA hs, RoPE on DVE/ACT (sign-folded sin table so rotate_half
    is a free-dim swap), stage2 kv = x_hp^T @ x_hp accumulated in PSUM
  - A2(b): PE transpose x -> xT, stage1 qT = WT.T @ xT -> resident bf16
    qT.  A2 for b0..b3 is interleaved (lag 2) to keep the PE dense;
    A2 for b4..b7 is deferred until after the AllReduce is issued so all
    kv partials finish as early as possible.
  - one AllReduce of kv, read back on the scalar DMA queue, cast on DVE
  - B(b): out[m,e] = qT_chunk.T @ blockdiag(kv head-pair), DMA out bf16.
"""

import sys

if "/opt/trn_rl_repo" not in sys.path:
    sys.path.insert(0, "/opt/trn_rl_repo")

import numpy as np
import ml_dtypes

import concourse.bass as bass
import concourse.mybir as mybir
import concourse.tile as tile
from concourse import bacc
from concourse.masks import make_identity
from concourse.tile_rust import add_dep_helper

B = 8          # batch
N = 8192       # sequence
D = 512        # hidden
NCORES = 8
R = N // NCORES          # rows per core (1024)
CH = R // 128            # 128-row chunks per batch per core (8)
MB_CH = 4                # chunks per m-block
NMB = CH // MB_CH        # m-blocks per batch (2)
LAG = 2                  # batches stage1 trails stage2 by (early batches)
NEARLY = 4               # how many batches get interleaved A2
GROUP = 4                # batches per kv AllReduce
BF16 = mybir.dt.bfloat16
F32 = mybir.dt.float32

_CACHE: dict = {}


def _build():
    nc = bacc.Bacc(trn_type="TRN2", num_devices=NCORES)

    hs_ext = nc.declare_dram_parameter("hs", [B, R, D], BF16, isOutput=False)
    cos_ext = nc.declare_dram_parameter("cosb", [R, D], BF16, isOutput=False)
    sin_ext = nc.declare_dram_parameter("sins", [R, D], BF16, isOutput=False)
    wt_ext = nc.declare_dram_parameter("wt", [D, D], BF16, isOutput=False)
    out_ext = nc.declare_dram_parameter("out", [B, R, D], BF16, isOutput=True)

    kv_part = nc.dram_tensor("kv_part", [B, 4, 2, 64, 64], F32)
    kv_red = nc.dram_tensor("kv_red", [B, 4, 2, 64, 64], F32, addr_space="Shared")

    rg = [list(range(NCORES))]

    with tile.TileContext(nc) as tc:
        with (
            tc.tile_pool(name="singles", bufs=1) as singles,
            tc.tile_pool(name="hs_p", bufs=2) as hs_p,
            tc.tile_pool(name="x_p", bufs=6) as x_p,
            tc.tile_pool(name="xt_p", bufs=2) as xt_p,
            tc.tile_pool(name="kvs_p", bufs=2) as kvs_p,
            tc.tile_pool(name="out_p", bufs=3) as out_p,
            tc.tile_pool(name="tp_ps", bufs=2, space="PSUM") as tp_ps,
            tc.tile_pool(name="q_ps", bufs=2, space="PSUM") as q_ps,
            tc.tile_pool(name="kv_ps", bufs=1, space="PSUM") as kv_ps,
            tc.tile_pool(name="o_ps", bufs=3, space="PSUM") as o_ps,
        ):
            import os as _os
            if _os.environ.get("K_WARM", "0") == "1":
                # communicator warm-up: absorbs the one-time cross-core barrier
                warm_in = nc.dram_tensor("warm_in", [1, 32], F32)
                warm_out = nc.dram_tensor("warm_out", [1, 32], F32,
                                          addr_space="Shared")
                warm_sb = singles.tile([1, 32], F32, name="warm_sb")
                warm_sb2 = singles.tile([1, 32], F32, name="warm_sb2")
                nc.vector.memset(warm_sb, 0.0)
                nc.sync.dma_start(out=warm_in[:, :], in_=warm_sb)
                wc = nc.gpsimd.collective_compute(
                    "AllReduce", mybir.AluOpType.add, replica_groups=rg,
                    ins=[warm_in[:, :]], outs=[warm_out[:, :]])
                wrb = nc.sync.dma_start(out=warm_sb2, in_=warm_out[:, :])
                add_dep_helper(wrb.ins, wc.ins, reason="warm readback")

            # ---- resident inputs; split across the two HWDGE queues so the
            # first batch's hs DMA isn't stuck behind all the constants ----
            cos_sb = singles.tile([128, CH, D], BF16, name="cos_sb")
            sin_sb = singles.tile([128, CH, D], BF16, name="sin_sb")
            cos_r = cos_ext.rearrange("(c p) d -> p c d", p=128)
            sin_r = sin_ext.rearrange("(c p) d -> p c d", p=128)
            # batch-0 first slice goes ahead of the bulk constants so RoPE
            # and the first kv matmuls start as early as possible
            hs0_t = hs_p.tile([128, MB_CH, D], BF16, name="hs_t")
            nc.sync.dma_start(
                out=hs0_t,
                in_=hs_ext[0].rearrange("(c p) d -> p c d", p=128)[:, 0:MB_CH, :])
            nc.sync.dma_start(out=cos_sb[:, 0:MB_CH, :], in_=cos_r[:, 0:MB_CH, :])
            nc.scalar.dma_start(out=sin_sb[:, 0:MB_CH, :], in_=sin_r[:, 0:MB_CH, :])
            nc.sync.dma_start(out=cos_sb[:, MB_CH:CH, :], in_=cos_r[:, MB_CH:CH, :])
            nc.scalar.dma_start(out=sin_sb[:, MB_CH:CH, :], in_=sin_r[:, MB_CH:CH, :])

            # WT strips (host-transposed): wt_sb[:, db, e] = W[e, db*128+p]
            wt_sb = singles.tile([128, 4, D], BF16, name="wt_sb")
            nc.scalar.dma_start(out=wt_sb,
                                in_=wt_ext.rearrange("(b p) e -> p b e", p=128))

            ident = singles.tile([128, 128], BF16, name="ident")
            make_identity(nc, ident)

            qT_sb = singles.tile([128, 4, B * R], BF16, name="qT_sb")
            kvblk = singles.tile([128, B, 4, 128], BF16, name="kvblk")
            nc.gpsimd.memset(kvblk, 0.0)

            x_tiles = {}
            kv_writers = []

            def emit_a1(b, hs_pre=None):
                """DMA + RoPE + stage2 kv accumulation for batch b."""
                x_t = x_p.tile([128, CH, D], BF16, name="x_t")
                x_tiles[b] = x_t
                kvp = kv_ps.tile([128, 4, 128], F32, name="kvp")
                hs_r = hs_ext[b].rearrange("(c p) d -> p c d", p=128)
                for mb in range(NMB):
                    cs = slice(mb * MB_CH, (mb + 1) * MB_CH)
                    if mb == 0 and hs_pre is not None:
                        hs_t = hs_pre
                    else:
                        hs_t = hs_p.tile([128, MB_CH, D], BF16, name="hs_t")
                        nc.sync.dma_start(out=hs_t, in_=hs_r[:, cs, :])
                    # RoPE: x = hs*cos + swap_half(hs)*sin_signed
                    nc.any.tensor_tensor(
                        x_t[:, cs, 0:256], hs_t[:, :, 256:512], sin_sb[:, cs, 0:256],
                        mybir.AluOpType.mult)
                    nc.any.tensor_tensor(
                        x_t[:, cs, 256:512], hs_t[:, :, 0:256], sin_sb[:, cs, 256:512],
                        mybir.AluOpType.mult)
                    nc.vector.tensor_tensor(hs_t, hs_t, cos_sb[:, cs, :],
                                            mybir.AluOpType.mult)
                    nc.vector.tensor_tensor(x_t[:, cs, :], x_t[:, cs, :], hs_t,
                                            mybir.AluOpType.add)
                    for cc in range(MB_CH):
                        c = mb * MB_CH + cc
                        for hp in range(4):
                            xs = x_t[:, c, hp * 128:(hp + 1) * 128]
                            nc.tensor.matmul(
                                kvp[:, hp, :], xs, xs,
                                start=(c == 0 and hp == 0),
                                stop=(c == CH - 1 and hp == 3))
                # evacuate kv partial (diagonal 64x64 blocks)
                kv_sb = kvs_p.tile([128, 4, 128], F32, name="kv_sb")
                nc.any.tensor_copy(out=kv_sb, in_=kvp)
                d0 = nc.sync.dma_start(
                    out=kv_part[b, :, 0].rearrange("h d e -> d h e"),
                    in_=kv_sb[0:64, :, 0:64])
                d1 = nc.sync.dma_start(
                    out=kv_part[b, :, 1].rearrange("h d e -> d h e"),
                    in_=kv_sb[64:128, :, 64:128])
                kv_writers.extend([d0, d1])

            def emit_a2(b):
                """PE transposes + stage1 qT for batch b."""
                x_t = x_tiles.pop(b)
                for mb in range(NMB):
                    xt_t = xt_p.tile([128, 4, MB_CH * 128], BF16, name="xt_t")
                    for cc in range(MB_CH):
                        c = mb * MB_CH + cc
                        tp = tp_ps.tile([128, 4, 128], BF16, name="tp")
                        for db in range(4):
                            nc.tensor.transpose(
                                tp[:, db, :], x_t[:, c, db * 128:(db + 1) * 128], ident)
                        nc.any.tensor_copy(out=xt_t[:, :, cc * 128:(cc + 1) * 128],
                                           in_=tp)
                    for eb in range(4):
                        qp = q_ps.tile([128, MB_CH * 128], F32, name="qp")
                        for db in range(4):
                            nc.tensor.matmul(
                                qp, wt_sb[:, db, eb * 128:(eb + 1) * 128],
                                xt_t[:, db, :],
                                start=(db == 0), stop=(db == 3))
                        nc.any.tensor_copy(
                            out=qT_sb[:, eb, b * R + mb * MB_CH * 128:
                                      b * R + (mb + 1) * MB_CH * 128],
                            in_=qp)

            colls = []

            def emit_allreduce(g0, g1):
                coll = nc.gpsimd.collective_compute(
                    "AllReduce", mybir.AluOpType.add, replica_groups=rg,
                    ins=[kv_part[g0:g1]], outs=[kv_red[g0:g1]])
                for w in kv_writers:
                    add_dep_helper(coll.ins, w.ins, reason="allreduce after kv dma")
                kv_writers.clear()
                colls.append((g0, g1, coll))

            def emit_readbacks():
                # SWDGE cast-DMAs (fp32 -> bf16) straight into the
                # block-diagonal tiles; emitted after both collectives so the
                # in-order gpsimd queue never delays an AllReduce trigger
                for g0, g1, coll in colls:
                    for bb in range(g0, g1):
                        r0 = nc.gpsimd.dma_start(
                            out=kvblk[0:64, bb, :, 0:64],
                            in_=kv_red[bb, :, 0].rearrange("h d e -> d h e"))
                        r1 = nc.gpsimd.dma_start(
                            out=kvblk[64:128, bb, :, 64:128],
                            in_=kv_red[bb, :, 1].rearrange("h d e -> d h e"))
                        add_dep_helper(r0.ins, coll.ins, reason="rb after allreduce")
                        add_dep_helper(r1.ins, coll.ins, reason="rb after allreduce")

            # ---------------- phase A ----------------
            for b in range(B):
                emit_a1(b, hs_pre=hs0_t if b == 0 else None)
                if LAG <= b < NEARLY + LAG:
                    emit_a2(b - LAG)
                if b == GROUP - 1:
                    emit_allreduce(0, GROUP)
                elif b == B - 1:
                    emit_allreduce(GROUP, B)
            emit_readbacks()

            # deferred stage1 for the late batches fills the AllReduce window
            for b in range(NEARLY, B):
                emit_a2(b)

            # ---------------- phase B ----------------
            for b in range(B):
                out_r = out_ext[b].rearrange("(c p) d -> p c d", p=128)
                out_sb = out_p.tile([128, CH, D], BF16, name="out_sb")
                for c in range(CH):
                    op = o_ps.tile([128, D], F32, name="op")
                    for hp in range(4):
                        nc.tensor.matmul(
                            op[:, hp * 128:(hp + 1) * 128],
                            qT_sb[:, hp, b * R + c * 128:b * R + (c + 1) * 128],
                            kvblk[:, b, hp, :],
                            start=(hp == 0), stop=(hp == 3))
                    if c % 2 == 0:
                        nc.vector.tensor_copy(out=out_sb[:, c, :], in_=op)
                    else:
                        nc.scalar.copy(out=out_sb[:, c, :], in_=op)
                nc.sync.dma_start(out=out_r, in_=out_sb)

    nc.compile()
    return nc


def _prep_in_maps(hidden_states, W, cos, sin):
    bf16 = ml_dtypes.bfloat16
    hs = np.ascontiguousarray(hidden_states, dtype=np.float32)
    cos = np.asarray(cos, dtype=np.float32)
    sin = np.asarray(sin, dtype=np.float32)
    sin_signed = np.concatenate([-sin[:, : D // 2], sin[:, D // 2:]], axis=1)
    wt16 = np.ascontiguousarray(np.asarray(W, dtype=np.float32).T).astype(bf16)
    in_maps = []
    for c in range(NCORES):
        rows = slice(c * R, (c + 1) * R)
        in_maps.append({
            "hs": np.ascontiguousarray(hs[:, rows, :]).astype(bf16),
            "cosb": np.ascontiguousarray(cos[rows]).astype(bf16),
            "sins": np.ascontiguousarray(sin_signed[rows]).astype(bf16),
            "wt": wt16,
        })
    return in_maps


def _collect(results):
    out = np.empty((B, N, D), dtype=np.float32)
    for c in range(NCORES):
        out[:, c * R:(c + 1) * R, :] = results[c]["out"].astype(np.float32)
    return out


def kernel(hidden_states, W, cos, sin):
    from concourse.bass_utils import run_bass_kernel_spmd

    nc = _CACHE.get("nc")
    if nc is None:
        nc = _build()
        _CACHE["nc"] = nc

    in_maps = _prep_in_maps(hidden_states, W, cos, sin)
    res = run_bass_kernel_spmd(nc, in_maps, list(range(NCORES)))
    return _collect(res.results)


# revision 17
# speedup vs baseline: 1.1267x; 1.0633x over previous
"""Trainium2 Bass kernel for nn_DenseAttention_85074712199779.

reference computation (B=8, N=8192, D=512, H=8, DH=64):
    x   = hs * cos + rotate_half(hs) * sin          # RoPE
    q   = x @ W.T                                   # dense projection
    kv  = einsum('bnhd,bnhe->bhde', xh, xh)         # per-head K^T V
    out = einsum('bnhd,bhde->bnhe', qh, kv)         # per-head Q (K^T V)

Sharding: sequence dim N split across the 8 NeuronCores (1024 rows/core,
all batches).  kv needs a cross-core sum -> one small fp32 AllReduce
(1 MB) that overlaps the q-projection matmuls.  All matmuls run in bf16
(fp32 PSUM accumulation); inputs are pre-cast to bf16 on the host (also
halves HBM traffic).  Output is written bf16 and upcast on the host.

Device schedule per core (b = batch):
  - A1(b): DMA hs, RoPE on DVE/ACT (sign-folded sin table so rotate_half
    is a free-dim swap), stage2 kv = x_hp^T @ x_hp accumulated in PSUM
  - A2(b): PE transpose x -> xT, stage1 qT = WT.T @ xT -> resident bf16
    qT.  A2 for b0..b3 is interleaved (lag 2) to keep the PE dense;
    A2 for b4..b7 is deferred until after the AllReduce is issued so all
    kv partials finish as early as possible.
  - one AllReduce of kv, read back on the scalar DMA queue, cast on DVE
  - B(b): out[m,e] = qT_chunk.T @ blockdiag(kv head-pair), DMA out bf16.
"""

import sys

if "/opt/trn_rl_repo" not in sys.path:
    sys.path.insert(0, "/opt/trn_rl_repo")

import numpy as np
import ml_dtypes

import concourse.bass as bass
import concourse.mybir as mybir
import concourse.tile as tile
from concourse import bacc
from concourse.masks import make_identity
from concourse.tile_rust import add_dep_helper

B = 8          # batch
N = 8192       # sequence
D = 512        # hidden
NCORES = 8
R = N // NCORES          # rows per core (1024)
CH = R // 128            # 128-row chunks per batch per core (8)
MB_CH = 4                # chunks per m-block
NMB = CH // MB_CH        # m-blocks per batch (2)
LAG = 2                  # batches stage1 trails stage2 by (early batches)
NEARLY = 4               # how many batches get interleaved A2
GROUP = 4                # batches per kv AllReduce
BF16 = mybir.dt.bfloat16
F32 = mybir.dt.float32

_CACHE: dict = {}


def _build():
    nc = bacc.Bacc(trn_type="TRN2", num_devices=NCORES)

    hs_ext = nc.declare_dram_parameter("hs", [B, R, D], BF16, isOutput=False)
    cos_ext = nc.declare_dram_parameter("cosb", [R, D], BF16, isOutput=False)
    sin_ext = nc.declare_dram_parameter("sins", [R, D], BF16, isOutput=False)
    wt_ext = nc.declare_dram_parameter("wt", [D, D], BF16, isOutput=False)
    out_ext = nc.declare_dram_parameter("out", [B, R, D], BF16, isOutput=True)

    kv_part = nc.dram_tensor("kv_part", [B, 4, 2, 64, 64], BF16)
    kv_red = nc.dram_tensor("kv_red", [B, 4, 2, 64, 64], BF16, addr_space="Shared")

    rg = [list(range(NCORES))]

    with tile.TileContext(nc) as tc:
        with (
            tc.tile_pool(name="singles", bufs=1) as singles,
            tc.tile_pool(name="hs_p", bufs=2) as hs_p,
            tc.tile_pool(name="x_p", bufs=6) as x_p,
            tc.tile_pool(name="xt_p", bufs=2) as xt_p,
            tc.tile_pool(name="kvs_p", bufs=2) as kvs_p,
            tc.tile_pool(name="out_p", bufs=3) as out_p,
            tc.tile_pool(name="tp_ps", bufs=2, space="PSUM") as tp_ps,
            tc.tile_pool(name="q_ps", bufs=2, space="PSUM") as q_ps,
            tc.tile_pool(name="kv_ps", bufs=1, space="PSUM") as kv_ps,
            tc.tile_pool(name="o_ps", bufs=3, space="PSUM") as o_ps,
        ):
            import os as _os
            if _os.environ.get("K_WARM", "0") == "1":
                # communicator warm-up: absorbs the one-time cross-core barrier
                warm_in = nc.dram_tensor("warm_in", [1, 32], F32)
                warm_out = nc.dram_tensor("warm_out", [1, 32], F32,
                                          addr_space="Shared")
                warm_sb = singles.tile([1, 32], F32, name="warm_sb")
                warm_sb2 = singles.tile([1, 32], F32, name="warm_sb2")
                nc.vector.memset(warm_sb, 0.0)
                nc.sync.dma_start(out=warm_in[:, :], in_=warm_sb)
                wc = nc.gpsimd.collective_compute(
                    "AllReduce", mybir.AluOpType.add, replica_groups=rg,
                    ins=[warm_in[:, :]], outs=[warm_out[:, :]])
                wrb = nc.sync.dma_start(out=warm_sb2, in_=warm_out[:, :])
                add_dep_helper(wrb.ins, wc.ins, reason="warm readback")

            # ---- resident inputs; split across the two HWDGE queues so the
            # first batch's hs DMA isn't stuck behind all the constants ----
            cos_sb = singles.tile([128, CH, D], BF16, name="cos_sb")
            sin_sb = singles.tile([128, CH, D], BF16, name="sin_sb")
            cos_r = cos_ext.rearrange("(c p) d -> p c d", p=128)
            sin_r = sin_ext.rearrange("(c p) d -> p c d", p=128)
            # batch-0 first slice goes ahead of the bulk constants so RoPE
            # and the first kv matmuls start as early as possible
            hs0_t = hs_p.tile([128, MB_CH, D], BF16, name="hs_t")
            nc.sync.dma_start(
                out=hs0_t,
                in_=hs_ext[0].rearrange("(c p) d -> p c d", p=128)[:, 0:MB_CH, :])
            nc.sync.dma_start(out=cos_sb[:, 0:MB_CH, :], in_=cos_r[:, 0:MB_CH, :])
            nc.scalar.dma_start(out=sin_sb[:, 0:MB_CH, :], in_=sin_r[:, 0:MB_CH, :])
            nc.sync.dma_start(out=cos_sb[:, MB_CH:CH, :], in_=cos_r[:, MB_CH:CH, :])
            nc.scalar.dma_start(out=sin_sb[:, MB_CH:CH, :], in_=sin_r[:, MB_CH:CH, :])

            # WT strips (host-transposed): wt_sb[:, db, e] = W[e, db*128+p]
            wt_sb = singles.tile([128, 4, D], BF16, name="wt_sb")
            nc.scalar.dma_start(out=wt_sb,
                                in_=wt_ext.rearrange("(b p) e -> p b e", p=128))

            ident = singles.tile([128, 128], BF16, name="ident")
            make_identity(nc, ident)

            qT_sb = singles.tile([128, 4, B * R], BF16, name="qT_sb")
            kvblk = singles.tile([128, B, 4, 128], BF16, name="kvblk")
            nc.gpsimd.memset(kvblk, 0.0)

            x_tiles = {}
            kv_writers = []

            def emit_a1(b, hs_pre=None):
                """DMA + RoPE + stage2 kv accumulation for batch b."""
                x_t = x_p.tile([128, CH, D], BF16, name="x_t")
                x_tiles[b] = x_t
                kvp = kv_ps.tile([128, 4, 128], F32, name="kvp")
                hs_r = hs_ext[b].rearrange("(c p) d -> p c d", p=128)
                for mb in range(NMB):
                    cs = slice(mb * MB_CH, (mb + 1) * MB_CH)
                    if mb == 0 and hs_pre is not None:
                        hs_t = hs_pre
                    else:
                        hs_t = hs_p.tile([128, MB_CH, D], BF16, name="hs_t")
                        nc.sync.dma_start(out=hs_t, in_=hs_r[:, cs, :])
                    # RoPE: x = hs*cos + swap_half(hs)*sin_signed
                    nc.any.tensor_tensor(
                        x_t[:, cs, 0:256], hs_t[:, :, 256:512], sin_sb[:, cs, 0:256],
                        mybir.AluOpType.mult)
                    nc.any.tensor_tensor(
                        x_t[:, cs, 256:512], hs_t[:, :, 0:256], sin_sb[:, cs, 256:512],
                        mybir.AluOpType.mult)
                    nc.vector.tensor_tensor(hs_t, hs_t, cos_sb[:, cs, :],
                                            mybir.AluOpType.mult)
                    nc.vector.tensor_tensor(x_t[:, cs, :], x_t[:, cs, :], hs_t,
                                            mybir.AluOpType.add)
                    for cc in range(MB_CH):
                        c = mb * MB_CH + cc
                        for hp in range(4):
                            xs = x_t[:, c, hp * 128:(hp + 1) * 128]
                            nc.tensor.matmul(
                                kvp[:, hp, :], xs, xs,
                                start=(c == 0 and hp == 0),
                                stop=(c == CH - 1 and hp == 3))
                # evacuate kv partial (diagonal 64x64 blocks)
                kv_sb = kvs_p.tile([128, 4, 128], BF16, name="kv_sb")
                nc.any.tensor_copy(out=kv_sb, in_=kvp)
                d0 = nc.sync.dma_start(
                    out=kv_part[b, :, 0].rearrange("h d e -> d h e"),
                    in_=kv_sb[0:64, :, 0:64])
                d1 = nc.sync.dma_start(
                    out=kv_part[b, :, 1].rearrange("h d e -> d h e"),
                    in_=kv_sb[64:128, :, 64:128])
                kv_writers.extend([d0, d1])

            def emit_a2(b):
                """PE transposes + stage1 qT for batch b."""
                x_t = x_tiles.pop(b)
                for mb in range(NMB):
                    xt_t = xt_p.tile([128, 4, MB_CH * 128], BF16, name="xt_t")
                    for cc in range(MB_CH):
                        c = mb * MB_CH + cc
                        tp = tp_ps.tile([128, 4, 128], BF16, name="tp")
                        for db in range(4):
                            nc.tensor.transpose(
                                tp[:, db, :], x_t[:, c, db * 128:(db + 1) * 128], ident)
                        nc.any.tensor_copy(out=xt_t[:, :, cc * 128:(cc + 1) * 128],
                                           in_=tp)
                    for eb in range(4):
                        qp = q_ps.tile([128, MB_CH * 128], F32, name="qp")
                        for db in range(4):
                            nc.tensor.matmul(
                                qp, wt_sb[:, db, eb * 128:(eb + 1) * 128],
                                xt_t[:, db, :],
                                start=(db == 0), stop=(db == 3))
                        nc.any.tensor_copy(
                            out=qT_sb[:, eb, b * R + mb * MB_CH * 128:
                                      b * R + (mb + 1) * MB_CH * 128],
                            in_=qp)

            colls = []

            def emit_allreduce(g0, g1):
                coll = nc.gpsimd.collective_compute(
                    "AllReduce", mybir.AluOpType.add, replica_groups=rg,
                    ins=[kv_part[g0:g1]], outs=[kv_red[g0:g1]])
                for w in kv_writers:
                    add_dep_helper(coll.ins, w.ins, reason="allreduce after kv dma")
                kv_writers.clear()
                colls.append((g0, g1, coll))

            def emit_readbacks():
                # SWDGE cast-DMAs (fp32 -> bf16) straight into the
                # block-diagonal tiles; emitted after both collectives so the
                # in-order gpsimd queue never delays an AllReduce trigger
                for g0, g1, coll in colls:
                    r0 = nc.scalar.dma_start(
                        out=kvblk[0:64, g0:g1, :, 0:64],
                        in_=kv_red[g0:g1, :, 0].rearrange("b h d e -> d b h e"))
                    r1 = nc.scalar.dma_start(
                        out=kvblk[64:128, g0:g1, :, 64:128],
                        in_=kv_red[g0:g1, :, 1].rearrange("b h d e -> d b h e"))
                    add_dep_helper(r0.ins, coll.ins, reason="rb after allreduce")
                    add_dep_helper(r1.ins, coll.ins, reason="rb after allreduce")

            # ---------------- phase A ----------------
            for b in range(B):
                emit_a1(b, hs_pre=hs0_t if b == 0 else None)
                if LAG <= b < NEARLY + LAG:
                    emit_a2(b - LAG)
                if b == GROUP - 1:
                    emit_allreduce(0, GROUP)
                elif b == B - 1:
                    emit_allreduce(GROUP, B)
            emit_readbacks()

            # deferred stage1 for the late batches fills the AllReduce window
            for b in range(NEARLY, B):
                emit_a2(b)

            # ---------------- phase B ----------------
            for b in range(B):
                out_r = out_ext[b].rearrange("(c p) d -> p c d", p=128)
                out_sb = out_p.tile([128, CH, D], BF16, name="out_sb")
                for c in range(CH):
                    op = o_ps.tile([128, D], F32, name="op")
                    for hp in range(4):
                        nc.tensor.matmul(
                            op[:, hp * 128:(hp + 1) * 128],
                            qT_sb[:, hp, b * R + c * 128:b * R + (c + 1) * 128],
                            kvblk[:, b, hp, :],
                            start=(hp == 0), stop=(hp == 3))
                    if c % 2 == 0:
                        nc.vector.tensor_copy(out=out_sb[:, c, :], in_=op)
                    else:
                        nc.scalar.copy(out=out_sb[:, c, :], in_=op)
                nc.sync.dma_start(out=out_r, in_=out_sb)

    nc.compile()
    return nc


def _prep_in_maps(hidden_states, W, cos, sin):
    bf16 = ml_dtypes.bfloat16
    hs = np.ascontiguousarray(hidden_states, dtype=np.float32)
    cos = np.asarray(cos, dtype=np.float32)
    sin = np.asarray(sin, dtype=np.float32)
    sin_signed = np.concatenate([-sin[:, : D // 2], sin[:, D // 2:]], axis=1)
    wt16 = np.ascontiguousarray(np.asarray(W, dtype=np.float32).T).astype(bf16)
    in_maps = []
    for c in range(NCORES):
        rows = slice(c * R, (c + 1) * R)
        in_maps.append({
            "hs": np.ascontiguousarray(hs[:, rows, :]).astype(bf16),
            "cosb": np.ascontiguousarray(cos[rows]).astype(bf16),
            "sins": np.ascontiguousarray(sin_signed[rows]).astype(bf16),
            "wt": wt16,
        })
    return in_maps


def _collect(results):
    out = np.empty((B, N, D), dtype=np.float32)
    for c in range(NCORES):
        out[:, c * R:(c + 1) * R, :] = results[c]["out"].astype(np.float32)
    return out


def kernel(hidden_states, W, cos, sin):
    from concourse.bass_utils import run_bass_kernel_spmd

    nc = _CACHE.get("nc")
    if nc is None:
        nc = _build()
        _CACHE["nc"] = nc

    in_maps = _prep_in_maps(hidden_states, W, cos, sin)
    res = run_bass_kernel_spmd(nc, in_maps, list(range(NCORES)))
    return _collect(res.results)


# revision 18
# speedup vs baseline: 1.1642x; 1.0333x over previous
"""Trainium2 Bass kernel for nn_DenseAttention_85074712199779.

reference computation (B=8, N=8192, D=512, H=8, DH=64):
    x   = hs * cos + rotate_half(hs) * sin          # RoPE
    q   = x @ W.T                                   # dense projection
    kv  = einsum('bnhd,bnhe->bhde', xh, xh)         # per-head K^T V
    out = einsum('bnhd,bhde->bnhe', qh, kv)         # per-head Q (K^T V)

Sharding: sequence dim N split across the 8 NeuronCores (1024 rows/core,
all batches).  kv needs a cross-core sum -> one small fp32 AllReduce
(1 MB) that overlaps the q-projection matmuls.  All matmuls run in bf16
(fp32 PSUM accumulation); inputs are pre-cast to bf16 on the host (also
halves HBM traffic).  Output is written bf16 and upcast on the host.

Device schedule per core (b = batch):
  - A1(b): DMA hs, RoPE on DVE/ACT (sign-folded sin table so rotate_half
    is a free-dim swap), stage2 kv = x_hp^T @ x_hp accumulated in PSUM
  - A2(b): PE transpose x -> xT, stage1 qT = WT.T @ xT -> resident bf16
    qT.  A2 for b0..b3 is interleaved (lag 2) to keep the PE dense;
    A2 for b4..b7 is deferred until after the AllReduce is issued so all
    kv partials finish as early as possible.
  - one AllReduce of kv, read back on the scalar DMA queue, cast on DVE
  - B(b): out[m,e] = qT_chunk.T @ blockdiag(kv head-pair), DMA out bf16.
"""

import sys

if "/opt/trn_rl_repo" not in sys.path:
    sys.path.insert(0, "/opt/trn_rl_repo")

import numpy as np
import ml_dtypes

import concourse.bass as bass
import concourse.mybir as mybir
import concourse.tile as tile
from concourse import bacc
from concourse.masks import make_identity
from concourse.tile_rust import add_dep_helper

B = 8          # batch
N = 8192       # sequence
D = 512        # hidden
NCORES = 8
R = N // NCORES          # rows per core (1024)
CH = R // 128            # 128-row chunks per batch per core (8)
MB_CH = 4                # chunks per m-block
NMB = CH // MB_CH        # m-blocks per batch (2)
LAG = 2                  # batches stage1 trails stage2 by (early batches)
NEARLY = 3               # how many batches get interleaved A2
GROUP = 4                # batches per kv AllReduce
BF16 = mybir.dt.bfloat16
F32 = mybir.dt.float32

_CACHE: dict = {}


def _build():
    nc = bacc.Bacc(trn_type="TRN2", num_devices=NCORES)

    hs_ext = nc.declare_dram_parameter("hs", [B, R, D], BF16, isOutput=False)
    cos_ext = nc.declare_dram_parameter("cosb", [R, D], BF16, isOutput=False)
    sin_ext = nc.declare_dram_parameter("sins", [R, D], BF16, isOutput=False)
    wt_ext = nc.declare_dram_parameter("wt", [D, D], BF16, isOutput=False)
    out_ext = nc.declare_dram_parameter("out", [B, R, D], BF16, isOutput=True)

    kv_part = nc.dram_tensor("kv_part", [B, 4, 2, 64, 64], BF16)
    kv_red = nc.dram_tensor("kv_red", [B, 4, 2, 64, 64], BF16, addr_space="Shared")

    rg = [list(range(NCORES))]

    with tile.TileContext(nc) as tc:
        with (
            tc.tile_pool(name="singles", bufs=1) as singles,
            tc.tile_pool(name="hs_p", bufs=2) as hs_p,
            tc.tile_pool(name="x_p", bufs=6) as x_p,
            tc.tile_pool(name="xt_p", bufs=2) as xt_p,
            tc.tile_pool(name="kvs_p", bufs=2) as kvs_p,
            tc.tile_pool(name="out_p", bufs=3) as out_p,
            tc.tile_pool(name="tp_ps", bufs=2, space="PSUM") as tp_ps,
            tc.tile_pool(name="q_ps", bufs=2, space="PSUM") as q_ps,
            tc.tile_pool(name="kv_ps", bufs=1, space="PSUM") as kv_ps,
            tc.tile_pool(name="o_ps", bufs=3, space="PSUM") as o_ps,
        ):
            import os as _os
            if _os.environ.get("K_WARM", "0") == "1":
                # communicator warm-up: absorbs the one-time cross-core barrier
                warm_in = nc.dram_tensor("warm_in", [1, 32], F32)
                warm_out = nc.dram_tensor("warm_out", [1, 32], F32,
                                          addr_space="Shared")
                warm_sb = singles.tile([1, 32], F32, name="warm_sb")
                warm_sb2 = singles.tile([1, 32], F32, name="warm_sb2")
                nc.vector.memset(warm_sb, 0.0)
                nc.sync.dma_start(out=warm_in[:, :], in_=warm_sb)
                wc = nc.gpsimd.collective_compute(
                    "AllReduce", mybir.AluOpType.add, replica_groups=rg,
                    ins=[warm_in[:, :]], outs=[warm_out[:, :]])
                wrb = nc.sync.dma_start(out=warm_sb2, in_=warm_out[:, :])
                add_dep_helper(wrb.ins, wc.ins, reason="warm readback")

            # ---- resident inputs; split across the two HWDGE queues so the
            # first batch's hs DMA isn't stuck behind all the constants ----
            cos_sb = singles.tile([128, CH, D], BF16, name="cos_sb")
            sin_sb = singles.tile([128, CH, D], BF16, name="sin_sb")
            cos_r = cos_ext.rearrange("(c p) d -> p c d", p=128)
            sin_r = sin_ext.rearrange("(c p) d -> p c d", p=128)
            # batch-0 first slice goes ahead of the bulk constants so RoPE
            # and the first kv matmuls start as early as possible
            hs0_t = hs_p.tile([128, MB_CH, D], BF16, name="hs_t")
            nc.sync.dma_start(
                out=hs0_t,
                in_=hs_ext[0].rearrange("(c p) d -> p c d", p=128)[:, 0:MB_CH, :])
            nc.sync.dma_start(out=cos_sb[:, 0:MB_CH, :], in_=cos_r[:, 0:MB_CH, :])
            nc.scalar.dma_start(out=sin_sb[:, 0:MB_CH, :], in_=sin_r[:, 0:MB_CH, :])
            nc.sync.dma_start(out=cos_sb[:, MB_CH:CH, :], in_=cos_r[:, MB_CH:CH, :])
            nc.scalar.dma_start(out=sin_sb[:, MB_CH:CH, :], in_=sin_r[:, MB_CH:CH, :])

            # WT strips (host-transposed): wt_sb[:, db, e] = W[e, db*128+p]
            wt_sb = singles.tile([128, 4, D], BF16, name="wt_sb")
            nc.scalar.dma_start(out=wt_sb,
                                in_=wt_ext.rearrange("(b p) e -> p b e", p=128))

            ident = singles.tile([128, 128], BF16, name="ident")
            make_identity(nc, ident)

            qT_sb = singles.tile([128, 4, B * R], BF16, name="qT_sb")
            kvblk = singles.tile([128, B, 4, 128], BF16, name="kvblk")
            nc.gpsimd.memset(kvblk, 0.0)

            x_tiles = {}
            kv_writers = []

            def emit_a1(b, hs_pre=None):
                """DMA + RoPE + stage2 kv accumulation for batch b."""
                x_t = x_p.tile([128, CH, D], BF16, name="x_t")
                x_tiles[b] = x_t
                kvp = kv_ps.tile([128, 4, 128], F32, name="kvp")
                hs_r = hs_ext[b].rearrange("(c p) d -> p c d", p=128)
                for mb in range(NMB):
                    cs = slice(mb * MB_CH, (mb + 1) * MB_CH)
                    if mb == 0 and hs_pre is not None:
                        hs_t = hs_pre
                    else:
                        hs_t = hs_p.tile([128, MB_CH, D], BF16, name="hs_t")
                        nc.sync.dma_start(out=hs_t, in_=hs_r[:, cs, :])
                    # RoPE: x = hs*cos + swap_half(hs)*sin_signed
                    nc.any.tensor_tensor(
                        x_t[:, cs, 0:256], hs_t[:, :, 256:512], sin_sb[:, cs, 0:256],
                        mybir.AluOpType.mult)
                    nc.any.tensor_tensor(
                        x_t[:, cs, 256:512], hs_t[:, :, 0:256], sin_sb[:, cs, 256:512],
                        mybir.AluOpType.mult)
                    nc.vector.tensor_tensor(hs_t, hs_t, cos_sb[:, cs, :],
                                            mybir.AluOpType.mult)
                    nc.vector.tensor_tensor(x_t[:, cs, :], x_t[:, cs, :], hs_t,
                                            mybir.AluOpType.add)
                    for cc in range(MB_CH):
                        c = mb * MB_CH + cc
                        for hp in range(4):
                            xs = x_t[:, c, hp * 128:(hp + 1) * 128]
                            nc.tensor.matmul(
                                kvp[:, hp, :], xs, xs,
                                start=(c == 0 and hp == 0),
                                stop=(c == CH - 1 and hp == 3))
                # evacuate kv partial (diagonal 64x64 blocks)
                kv_sb = kvs_p.tile([128, 4, 128], BF16, name="kv_sb")
                nc.any.tensor_copy(out=kv_sb, in_=kvp)
                d0 = nc.sync.dma_start(
                    out=kv_part[b, :, 0].rearrange("h d e -> d h e"),
                    in_=kv_sb[0:64, :, 0:64])
                d1 = nc.sync.dma_start(
                    out=kv_part[b, :, 1].rearrange("h d e -> d h e"),
                    in_=kv_sb[64:128, :, 64:128])
                kv_writers.extend([d0, d1])

            def emit_a2(b):
                """PE transposes + stage1 qT for batch b."""
                x_t = x_tiles.pop(b)
                for mb in range(NMB):
                    xt_t = xt_p.tile([128, 4, MB_CH * 128], BF16, name="xt_t")
                    for cc in range(MB_CH):
                        c = mb * MB_CH + cc
                        tp = tp_ps.tile([128, 4, 128], BF16, name="tp")
                        for db in range(4):
                            nc.tensor.transpose(
                                tp[:, db, :], x_t[:, c, db * 128:(db + 1) * 128], ident)
                        nc.any.tensor_copy(out=xt_t[:, :, cc * 128:(cc + 1) * 128],
                                           in_=tp)
                    for eb in range(4):
                        qp = q_ps.tile([128, MB_CH * 128], F32, name="qp")
                        for db in range(4):
                            nc.tensor.matmul(
                                qp, wt_sb[:, db, eb * 128:(eb + 1) * 128],
                                xt_t[:, db, :],
                                start=(db == 0), stop=(db == 3))
                        nc.any.tensor_copy(
                            out=qT_sb[:, eb, b * R + mb * MB_CH * 128:
                                      b * R + (mb + 1) * MB_CH * 128],
                            in_=qp)

            colls = []

            def emit_allreduce(g0, g1):
                coll = nc.gpsimd.collective_compute(
                    "AllReduce", mybir.AluOpType.add, replica_groups=rg,
                    ins=[kv_part[g0:g1]], outs=[kv_red[g0:g1]])
                for w in kv_writers:
                    add_dep_helper(coll.ins, w.ins, reason="allreduce after kv dma")
                kv_writers.clear()
                colls.append((g0, g1, coll))

            def emit_readbacks():
                # SWDGE cast-DMAs (fp32 -> bf16) straight into the
                # block-diagonal tiles; emitted after both collectives so the
                # in-order gpsimd queue never delays an AllReduce trigger
                for g0, g1, coll in colls:
                    r0 = nc.scalar.dma_start(
                        out=kvblk[0:64, g0:g1, :, 0:64],
                        in_=kv_red[g0:g1, :, 0].rearrange("b h d e -> d b h e"))
                    r1 = nc.scalar.dma_start(
                        out=kvblk[64:128, g0:g1, :, 64:128],
                        in_=kv_red[g0:g1, :, 1].rearrange("b h d e -> d b h e"))
                    add_dep_helper(r0.ins, coll.ins, reason="rb after allreduce")
                    add_dep_helper(r1.ins, coll.ins, reason="rb after allreduce")

            # ---------------- phase A ----------------
            for b in range(B):
                emit_a1(b, hs_pre=hs0_t if b == 0 else None)
                if LAG <= b < NEARLY + LAG:
                    emit_a2(b - LAG)
                if b == GROUP - 1:
                    emit_allreduce(0, GROUP)
                elif b == B - 1:
                    emit_allreduce(GROUP, B)
            emit_readbacks()

            # deferred stage1 for the late batches fills the AllReduce window
            for b in range(NEARLY, B):
                emit_a2(b)

            # ---------------- phase B ----------------
            for b in range(B):
                out_r = out_ext[b].rearrange("(c p) d -> p c d", p=128)
                out_sb = out_p.tile([128, CH, D], BF16, name="out_sb")
                for c in range(CH):
                    op = o_ps.tile([128, D], F32, name="op")
                    for hp in range(4):
                        nc.tensor.matmul(
                            op[:, hp * 128:(hp + 1) * 128],
                            qT_sb[:, hp, b * R + c * 128:b * R + (c + 1) * 128],
                            kvblk[:, b, hp, :],
                            start=(hp == 0), stop=(hp == 3))
                    if c % 2 == 0:
                        nc.vector.tensor_copy(out=out_sb[:, c, :], in_=op)
                    else:
                        nc.scalar.copy(out=out_sb[:, c, :], in_=op)
                nc.sync.dma_start(out=out_r, in_=out_sb)

    nc.compile()
    return nc


def _prep_in_maps(hidden_states, W, cos, sin):
    bf16 = ml_dtypes.bfloat16
    hs = np.ascontiguousarray(hidden_states, dtype=np.float32)
    cos = np.asarray(cos, dtype=np.float32)
    sin = np.asarray(sin, dtype=np.float32)
    sin_signed = np.concatenate([-sin[:, : D // 2], sin[:, D // 2:]], axis=1)
    wt16 = np.ascontiguousarray(np.asarray(W, dtype=np.float32).T).astype(bf16)
    in_maps = []
    for c in range(NCORES):
        rows = slice(c * R, (c + 1) * R)
        in_maps.append({
            "hs": np.ascontiguousarray(hs[:, rows, :]).astype(bf16),
            "cosb": np.ascontiguousarray(cos[rows]).astype(bf16),
            "sins": np.ascontiguousarray(sin_signed[rows]).astype(bf16),
            "wt": wt16,
        })
    return in_maps


def _collect(results):
    out = np.empty((B, N, D), dtype=np.float32)
    for c in range(NCORES):
        out[:, c * R:(c + 1) * R, :] = results[c]["out"].astype(np.float32)
    return out


def kernel(hidden_states, W, cos, sin):
    from concourse.bass_utils import run_bass_kernel_spmd

    nc = _CACHE.get("nc")
    if nc is None:
        nc = _build()
        _CACHE["nc"] = nc

    in_maps = _prep_in_maps(hidden_states, W, cos, sin)
    res = run_bass_kernel_spmd(nc, in_maps, list(range(NCORES)))
    return _collect(res.results)
